# revision 1
# baseline (speedup 1.0000x reference)
"""ClauseGCN (3-layer GCN with GraphNorm) on 8 Trainium2 NeuronCores.

Strategy:
  - Nodes are sharded across the 8 cores by graph id (batch is sorted, 256
    graphs -> 32 graphs per core).  Each core owns its nodes' rows of h and
    the edges whose DESTINATION lands on that core.
  - Per core, destination nodes are bin-packed into blocks of 128 ("dest
    blocks").  The sparse aggregation for one block is computed on the
    TensorEngine as sum_t G_t^T @ S_t accumulated in PSUM, where G_t is a
    [128 edge, 128 feat] tile of gathered source rows (dma_gather) and
    S_t[e, d] = val[e] * (dest_slot[e] == d) is built on the VectorEngine
    with a single fused tensor_scalar (is_equal then mult).
  - Source rows are fetched with the MoE dma_gather primitive (int16
    indices).  The gather table for layer 1 is a 384-row "combo" table
    (node embeddings are a pure function of (type, arity, argpos), each
    with a tiny integer domain, so embed+W_in is precomputed on-device
    into T[combo]).  For layers 2/3 the tables are the 8 per-core shards
    of the AllGather'ed h (each shard has <=32768 rows, fits int16).
  - GraphNorm is computed with one-hot matmuls against the 32 local graph
    slots (segment sums on the PE), using var = E[z^2] - a(2-a) mu^2.
  - h is AllGather'ed between layers (internal DRAM, gpsimd collective).
"""

import sys
import numpy as np

sys.path.insert(0, "/opt/trn_rl_repo")

import concourse.bass as bass
import concourse.bacc as bacc
import concourse.mybir as mybir
from concourse.tile import TileContext
from concourse.bass_utils import run_bass_kernel_spmd

F32 = mybir.dt.float32
I16 = mybir.dt.int16
OP = mybir.AluOpType
AF = mybir.ActivationFunctionType

NCORES = 8
HID = 128
L = 3
EPS = 1e-5
SIN = 8
NCOMBO = 6 * 5 * 10          # (type, arity, argpos) domains
TCOMBO = 384                 # padded combo-table rows
CH_TILES = 10                # gather-call chunk, in 128-slot tiles
NQ = 4                       # SWDGE queues


# ----------------------------------------------------------------------------
# host-side prep
# ----------------------------------------------------------------------------

def _quota(b, s):
    # tiles of 128 edge slots for cell (block b, source-core s); avg 2.5
    return 2 + ((b + s) & 1)


def _pack_core(node_deg8, n_nodes, B):
    """Assign this core's nodes to B blocks of <=128 nodes s.t. the edge count
    of cell (block, src_core) stays under _quota(b, s)*128.  node_deg8 is
    [n_nodes, 8] in-degree by source core.  Returns block id per node or None
    if packing failed."""
    caps = np.empty((B, 8), np.int64)
    for b in range(B):
        for s in range(8):
            caps[b, s] = _quota(b, s) * 128
    loads = np.zeros((B, 8), np.int64)
    counts = np.zeros(B, np.int64)
    order = np.argsort(-node_deg8.sum(1), kind="stable")
    assign = np.full(n_nodes, -1, np.int64)
    for n in order:
        d = node_deg8[n]
        slack = caps - loads - d[None, :]
        feas = (slack.min(1) >= 0) & (counts < 128)
        if not feas.any():
            return None
        score = np.where(feas, slack.min(1), -1)
        b = int(np.argmax(score))
        assign[n] = b
        loads[b] += d
        counts[b] += 1
    return assign


def _host_prep(x, edge_row, edge_col, edge_val, batch,
               W_in, b_in, Ws, bs, alphas, gammas, betas, G):
    N = x.shape[0]
    E = edge_row.shape[0]
    gpc = G // NCORES                       # graphs per core

    node_core = (batch.astype(np.int64) // gpc).clip(0, NCORES - 1)
    edge_core = node_core[edge_row]         # dest core owns the edge

    # ---- per-core in-degree by source core
    src_core = node_core[edge_col]

    # ---- decide block count B (uniform across cores)
    counts_n = np.bincount(node_core, minlength=NCORES)
    counts_e = np.bincount(edge_core, minlength=NCORES)
    B = 1
    for c in range(NCORES):
        B = max(B, int(np.ceil(counts_n[c] / 128.0)),
                int(np.ceil(counts_e[c] / 2100.0)))
    B = int(np.ceil(B / 8.0) * 8)           # multiple of 8 for chunking

    Npad = B * 128
    assert Npad <= 32768, "per-core shard must fit int16 gather indices"

    SL = B * 320                            # slots per stream (sum of quotas *128)
    CH = CH_TILES * 128
    while SL % CH != 0:
        B += 8
        Npad = B * 128
        SL = B * 320

    # ---- per-core packing and permutation
    perm = [None] * NCORES                  # local node id -> packed row
    nodes_of = [None] * NCORES
    for c in range(NCORES):
        nodes_c = np.nonzero(node_core == c)[0]
        n_c = len(nodes_c)
        lid = np.full(N, -1, np.int64)
        lid[nodes_c] = np.arange(n_c)
        deg8 = np.zeros((n_c, 8), np.int64)
        m = edge_core == c
        np.add.at(deg8, (lid[edge_row[m]], src_core[m]), 1)
        assign = None
        B_try = B
        while assign is None:
            assign = _pack_core(deg8, n_c, B_try)
            if assign is None:
                B_try += 8
        if B_try != B:
            # grow B globally and redo everything (rare)
            return _host_prep_with_B(x, edge_row, edge_col, edge_val, batch,
                                     W_in, b_in, Ws, bs, alphas, gammas, betas,
                                     G, B_try)
        # rows: stable order within block
        order = np.argsort(assign * 1000000 + np.arange(n_c), kind="stable")
        rows = np.empty(n_c, np.int64)
        blk_count = np.zeros(B, np.int64)
        for i in np.argsort(assign, kind="stable"):
            b = assign[i]
            rows[i] = b * 128 + blk_count[b]
            blk_count[b] += 1
        perm[c] = rows
        nodes_of[c] = nodes_c
    return _finish_prep(x, edge_row, edge_col, edge_val, batch, W_in, b_in,
                        Ws, bs, alphas, gammas, betas, G, B, perm, nodes_of,
                        node_core, src_core, edge_core)


def _host_prep_with_B(x, edge_row, edge_col, edge_val, batch, W_in, b_in,
                      Ws, bs, alphas, gammas, betas, G, B):
    # retry wrapper used when a core failed to pack at the initial B
    gpc = G // NCORES
    N = x.shape[0]
    node_core = (batch.astype(np.int64) // gpc).clip(0, NCORES - 1)
    src_core = node_core[edge_col]
    edge_core = node_core[edge_row]
    B = int(np.ceil(B / 8.0) * 8)
    SL = B * 320
    CH = CH_TILES * 128
    while SL % CH != 0:
        B += 8
        SL = B * 320
    perm = [None] * NCORES
    nodes_of = [None] * NCORES
    for c in range(NCORES):
        nodes_c = np.nonzero(node_core == c)[0]
        n_c = len(nodes_c)
        lid = np.full(N, -1, np.int64)
        lid[nodes_c] = np.arange(n_c)
        deg8 = np.zeros((n_c, 8), np.int64)
        m = edge_core == c
        np.add.at(deg8, (lid[edge_row[m]], src_core[m]), 1)
        assign = _pack_core(deg8, n_c, B)
        if assign is None:
            return _host_prep_with_B(x, edge_row, edge_col, edge_val, batch,
                                     W_in, b_in, Ws, bs, alphas, gammas,
                                     betas, G, B + 8)
        rows = np.empty(n_c, np.int64)
        blk_count = np.zeros(B, np.int64)
        for i in np.argsort(assign, kind="stable"):
            b = assign[i]
            rows[i] = b * 128 + blk_count[b]
            blk_count[b] += 1
        perm[c] = rows
        nodes_of[c] = nodes_c
    return _finish_prep(x, edge_row, edge_col, edge_val, batch, W_in, b_in,
                        Ws, bs, alphas, gammas, betas, G, B, perm, nodes_of,
                        node_core, src_core, edge_core)


def _wrap_idx16(flat):
    """MoE dma_gather index layout: [128, n/16] int16, 16-partition wrap
    replicated 8x down the partitions. flat length must be /16."""
    n = len(flat)
    blk = flat.reshape(n // 16, 16).T.astype(np.int16)   # [16, n/16]
    out = np.zeros((128, n // 16), np.int16)
    for k in range(8):
        out[16 * k:16 * (k + 1)] = blk
    return out


def _finish_prep(x, edge_row, edge_col, edge_val, batch, W_in, b_in,
                 Ws, bs, alphas, gammas, betas, G, B, perm, nodes_of,
                 node_core, src_core, edge_core):
    N = x.shape[0]
    gpc = G // NCORES
    Npad = B * 128
    SL = B * 320
    CH = CH_TILES * 128

    # global packed row of every node
    grow = np.empty(N, np.int64)
    for c in range(NCORES):
        grow[nodes_of[c]] = c * Npad + perm[c]

    # combo id per node (layer-1 gather indices)
    ty = np.clip(x[:, 0].astype(np.int64), 0, 5)
    ar = np.clip(x[:, 1].astype(np.int64), 0, 4)
    po = np.clip(x[:, 2].astype(np.int64), 0, 9)
    combo = ty * 50 + ar * 10 + po

    # ---- per-core edge streams
    in_maps = []
    meta = dict(B=B, Npad=Npad, SL=SL, CH=CH, gpc=gpc)

    # constant tiles (same for all cores)
    iota128 = np.tile(np.arange(128, dtype=np.float32), (128, 1))
    iota32t = np.tile(np.arange(32, dtype=np.float32), (128, 1))
    ident128 = np.eye(128, dtype=np.float32)

    # embedding combo table, transposed: [16, 384] (15 real rows)
    embT = np.zeros((16, TCOMBO), np.float32)
    cid = np.arange(NCOMBO)
    cty, car, cpo = cid // 50, (cid // 10) % 5, cid % 10
    for t in range(6):
        embT[t, cid] = (cty == t).astype(np.float32)
    embT[6, cid] = np.log1p(car.astype(np.float32))
    div_term = np.exp(np.arange(0, SIN, 2, dtype=np.float32)
                      * (-np.log(10000.0) / SIN))
    for k in range(SIN // 2):
        embT[7 + 2 * k, cid] = np.sin(cpo * div_term[k])
        embT[8 + 2 * k, cid] = np.cos(cpo * div_term[k])

    W_in_pad = np.zeros((16, HID), np.float32)
    W_in_pad[:15] = W_in
    b_in_rep = np.tile(b_in[None, :], (128, 1)).astype(np.float32)

    consts = dict(
        iota128=iota128, iota32t=iota32t, ident128=ident128,
        embT=embT, W_in_pad=W_in_pad, b_in_rep=b_in_rep,
    )
    for l in range(L):
        consts[f"W_{l}"] = Ws[l].astype(np.float32)
        consts[f"bs_rep_{l}"] = np.tile(bs[l][None, :], (128, 1)).astype(np.float32)
        consts[f"gamma_rep_{l}"] = np.tile(gammas[l][None, :], (32, 1)).astype(np.float32)
        consts[f"beta_rep_{l}"] = np.tile(betas[l][None, :], (128, 1)).astype(np.float32)
    meta["alphas"] = [float(a) for a in alphas]

    for c in range(NCORES):
        m = edge_core == c
        er = edge_row[m]
        ec = edge_col[m]
        ev = edge_val[m].astype(np.float32)
        drow = grow[er] - c * Npad          # packed row within this core
        dblk = drow // 128
        dslot = drow % 128
        scre = node_core[ec]                # stream per edge
        sidx = grow[ec] - scre * Npad       # int16 idx into source shard
        cidx = combo[ec]                    # layer-1 idx

        # slot layout: per stream, block-major, quota(b,s) tiles per cell
        idx16 = np.zeros(NCORES * SL, np.int64)
        idxC = np.zeros(NCORES * SL, np.int64)
        dest = np.zeros(NCORES * SL, np.float32)
        val = np.zeros(NCORES * SL, np.float32)
        # cell start offsets within each stream
        cell_off = np.zeros((B, NCORES), np.int64)
        quota_arr = np.zeros((B, NCORES), np.int64)
        for s in range(NCORES):
            off = 0
            for b in range(B):
                cell_off[b, s] = off
                quota_arr[b, s] = _quota(b, s)
                off += _quota(b, s) * 128
            assert off == SL
        key = dblk * NCORES + scre
        order = np.argsort(key, kind="stable")
        ks = key[order]
        starts = np.searchsorted(ks, np.arange(B * NCORES), side="left")
        rank = np.arange(len(order)) - starts[ks]
        bo = dblk[order]; so = scre[order]
        assert (rank < quota_arr[bo, so] * 128).all(), "cell overflow (packer bug)"
        flat = so * SL + cell_off[bo, so] + rank
        idx16[flat] = sidx[order]
        idxC[flat] = cidx[order]
        dest[flat] = dslot[order]
        val[flat] = ev[order]
        idx16 = idx16.reshape(NCORES, SL)
        idxC = idxC.reshape(NCORES, SL)
        dest = dest.reshape(NCORES, SL)
        val = val.reshape(NCORES, SL)

        # wrapped int16 index arrays per stream, chunked by CH
        idx16_w = np.zeros((NCORES, 128, SL // 16), np.int16)
        idxC_w = np.zeros((NCORES, 128, SL // 16), np.int16)
        for s in range(NCORES):
            idx16_w[s] = _wrap_idx16(idx16[s])
            idxC_w[s] = _wrap_idx16(idxC[s])

        # dest/val in [128 partitions, total-tile] col layout:
        # tile (b, s, q) at column  b*20 + cum_quota(s) + q ; partition = slot%128
        TPB = 20  # tiles per block (sum of quotas per block)
        dest_cols = np.zeros((128, B * TPB), np.float32)
        val_cols = np.zeros((128, B * TPB), np.float32)
        for b in range(B):
            col = b * TPB
            for s in range(NCORES):
                qn = _quota(b, s)
                o = cell_off[b, s]
                seg_d = dest[s, o:o + qn * 128].reshape(qn, 128).T
                seg_v = val[s, o:o + qn * 128].reshape(qn, 128).T
                dest_cols[:, col:col + qn] = seg_d
                val_cols[:, col:col + qn] = seg_v
                col += qn

        # graphnorm per-core data
        nodes_c = nodes_of[c]
        bl = np.full(Npad, 32.0, np.float32)      # dummy rows -> slot 32
        bl_rows = perm[c]
        bl[bl_rows] = (batch[nodes_c] - c * gpc).astype(np.float32)
        batchloc = bl.reshape(B, 128).T.copy()    # [128, B]
        cnt = np.zeros(32, np.float32)
        for g in range(32):
            cnt[g] = float(np.sum(batch[nodes_c] == c * gpc + g))
        invc = (1.0 / np.clip(cnt, 1.0, None)).reshape(32, 1).astype(np.float32)

        im = dict(
            idx16=idx16_w.reshape(NCORES * 128, SL // 16),
            idxC=idxC_w.reshape(NCORES * 128, SL // 16),
            dest_cols=dest_cols, val_cols=val_cols,
            batchloc=batchloc, invc=invc,
        )
        im.update(consts)
        in_maps.append(im)

    meta["TPB"] = 20
    return in_maps, meta, perm, nodes_of


# ----------------------------------------------------------------------------
# device program
# ----------------------------------------------------------------------------

def _build_program(meta):
    B = meta["B"]; Npad = meta["Npad"]; SL = meta["SL"]; CH = meta["CH"]
    TPB = meta["TPB"]
    alphas = meta["alphas"]
    n_chunks = SL // CH

    nc = bacc.Bacc(num_swdge_queues=NQ)

    # ---- IO
    t_in = {}
    for name, shape, dt in [
        ("idx16", [NCORES * 128, SL // 16], I16),
        ("idxC", [NCORES * 128, SL // 16], I16),
        ("dest_cols", [128, B * TPB], F32),
        ("val_cols", [128, B * TPB], F32),
        ("batchloc", [128, B], F32),
        ("invc", [32, 1], F32),
        ("iota128", [128, 128], F32),
        ("iota32t", [128, 32], F32),
        ("ident128", [128, 128], F32),
        ("embT", [16, TCOMBO], F32),
        ("W_in_pad", [16, HID], F32),
        ("b_in_rep", [128, HID], F32),
    ]:
        t_in[name] = nc.dram_tensor(name, shape, dt, kind="ExternalInput")
    for l in range(L):
        t_in[f"W_{l}"] = nc.dram_tensor(f"W_{l}", [HID, HID], F32, kind="ExternalInput")
        t_in[f"bs_rep_{l}"] = nc.dram_tensor(f"bs_rep_{l}", [128, HID], F32, kind="ExternalInput")
        t_in[f"gamma_rep_{l}"] = nc.dram_tensor(f"gamma_rep_{l}", [32, HID], F32, kind="ExternalInput")
        t_in[f"beta_rep_{l}"] = nc.dram_tensor(f"beta_rep_{l}", [128, HID], F32, kind="ExternalInput")

    out_h = nc.dram_tensor("out_h", [Npad, HID], F32, kind="ExternalOutput")

    T_tab = nc.dram_tensor("T_tab", [TCOMBO, HID], F32)
    z_dram = nc.dram_tensor("z_dram", [Npad, HID], F32)
    h_shard = [nc.dram_tensor(f"h_shard_{l}", [Npad, HID], F32) for l in range(L - 1)]
    h_full = [nc.dram_tensor(f"h_full_{l}", [NCORES * Npad, HID], F32,
                             addr_space="Shared") for l in range(L - 1)]

    with TileContext(nc) as tc:
        with tc.tile_pool(name="const", bufs=1) as cpool, \
             tc.tile_pool(name="gbuf", bufs=2) as gpool, \
             tc.tile_pool(name="work", bufs=3) as wpool, \
             tc.tile_pool(name="small", bufs=2) as spool, \
             tc.tile_pool(name="stat", bufs=1) as stpool, \
             tc.tile_pool(name="psA", bufs=2, space="PSUM") as psA, \
             tc.tile_pool(name="psB", bufs=1, space="PSUM") as psB, \
             tc.tile_pool(name="psS", bufs=1, space="PSUM") as psS:

            # ---- resident constants
            iota128 = cpool.tile([128, 128], F32)
            nc.sync.dma_start(iota128[:], t_in["iota128"][:])
            iota32t = cpool.tile([128, 32], F32)
            nc.sync.dma_start(iota32t[:], t_in["iota32t"][:])
            ident = cpool.tile([128, 128], F32)
            nc.sync.dma_start(ident[:], t_in["ident128"][:])

            batchloc = cpool.tile([128, B], F32)
            nc.sync.dma_start(batchloc[:], t_in["batchloc"][:])
            invc = cpool.tile([32, 1], F32)
            nc.sync.dma_start(invc[:], t_in["invc"][:])
            b_in_rep = cpool.tile([128, HID], F32)
            nc.sync.dma_start(b_in_rep[:], t_in["b_in_rep"][:])
            Wl_t = []
            bsl_t = []
            gml_t = []
            btl_t = []
            for l in range(L):
                w = cpool.tile([HID, HID], F32, tag=f"W{l}")
                nc.sync.dma_start(w[:], t_in[f"W_{l}"][:])
                Wl_t.append(w)
                bb = cpool.tile([128, HID], F32, tag=f"bs{l}")
                nc.sync.dma_start(bb[:], t_in[f"bs_rep_{l}"][:])
                bsl_t.append(bb)
                gm = cpool.tile([32, HID], F32, tag=f"gm{l}")
                nc.sync.dma_start(gm[:], t_in[f"gamma_rep_{l}"][:])
                gml_t.append(gm)
                bt = cpool.tile([128, HID], F32, tag=f"bt{l}")
                nc.sync.dma_start(bt[:], t_in[f"beta_rep_{l}"][:])
                btl_t.append(bt)

            # ---- build combo table T = emb @ W_in + b_in
            embT_t = cpool.tile([16, TCOMBO], F32)
            nc.sync.dma_start(embT_t[:], t_in["embT"][:])
            W_in_t = cpool.tile([16, HID], F32)
            nc.sync.dma_start(W_in_t[:], t_in["W_in_pad"][:])
            for cidx in range(TCOMBO // 128):
                ps = psA.tile([128, HID], F32, tag="agg")
                nc.tensor.matmul(ps[:], embT_t[:, cidx * 128:(cidx + 1) * 128],
                                 W_in_t[:], start=True, stop=True)
                tt = wpool.tile([128, HID], F32, tag="Tt")
                nc.vector.tensor_tensor(tt[:], ps[:], b_in_rep[:], OP.add)
                nc.sync.dma_start(T_tab[cidx * 128:(cidx + 1) * 128, :], tt[:])

            # ---- gather buffers (zero once so pads multiply clean data)
            Gb = []
            for s in range(NCORES):
                row = []
                for k in range(2):
                    g = gpool.tile([128, CH], F32, tag=f"G{s}_{k}")
                    nc.vector.memset(g[:], 0.0)
                    row.append(g)
                Gb.append(row)

            # cumulative tile index per stream at block starts
            cum = np.zeros((B + 1, NCORES), np.int64)
            for b in range(B):
                for s in range(NCORES):
                    cum[b + 1, s] = cum[b, s] + _quota(b, s)

            for l in range(L):
                # gather tables for this layer
                if l == 0:
                    tables = [T_tab[:]] * NCORES
                    idx_dram = t_in["idxC"]
                else:
                    hf = h_full[l - 1]
                    tables = [hf[s * Npad:(s + 1) * Npad, :] for s in range(NCORES)]
                    idx_dram = t_in["idx16"]

                psum_S1 = psS.tile([32, HID], F32, tag="S1")
                psum_S2 = psS.tile([32, HID], F32, tag="S2")

                # schedule: emit gather for (s, chunk) just before first use
                emitted = [[False] * n_chunks for _ in range(NCORES)]
                idx_tiles = [[None] * n_chunks for _ in range(NCORES)]

                def ensure_chunk(s, k):
                    if emitted[s][k]:
                        return
                    emitted[s][k] = True
                    it = spool.tile([128, CH // 16], I16, tag="idx")
                    nc.sync.dma_start(
                        it[:], idx_dram[s * 128:(s + 1) * 128,
                                        k * (CH // 16):(k + 1) * (CH // 16)])
                    g = Gb[s][k % 2]
                    nc.gpsimd.dma_gather(
                        g[:].rearrange("p (t f) -> p t f", f=HID),
                        tables[s], it[:], CH, CH, HID,
                        single_packet=False, queue_num=(s + k) % NQ,
                    )

                dvw = [None, None]
                for b in range(B):
                    if b % 8 == 0:
                        dw = wpool.tile([128, 8 * TPB], F32, tag="dw")
                        nc.sync.dma_start(
                            dw[:], t_in["dest_cols"][:, b * TPB:(b + 8) * TPB])
                        vw = wpool.tile([128, 8 * TPB], F32, tag="vw")
                        nc.sync.dma_start(
                            vw[:], t_in["val_cols"][:, b * TPB:(b + 8) * TPB])
                        dvw = [dw, vw]
                    dest_cols, val_cols = dvw
                    woff = (b % 8) * TPB
                    ps_agg = psA.tile([128, 128], F32, tag="agg")  # [f, d]
                    ntile = TPB
                    ti = 0
                    for s in range(NCORES):
                        qn = _quota(b, s)
                        for q in range(qn):
                            st = cum[b, s] + q          # stream tile index
                            k = st // CH_TILES
                            col = st % CH_TILES
                            ensure_chunk(s, k)
                            S = wpool.tile([128, 128], F32, tag="S")
                            nc.vector.tensor_scalar(
                                S[:], iota128[:],
                                dest_cols[:, woff + ti:woff + ti + 1],
                                val_cols[:, woff + ti:woff + ti + 1],
                                OP.is_equal, OP.mult)
                            G2 = Gb[s][k % 2]
                            nc.tensor.matmul(
                                ps_agg[:],
                                G2[:, col * HID:(col + 1) * HID],
                                S[:],
                                start=(ti == 0), stop=(ti == ntile - 1))
                            ti += 1
                    aggT = wpool.tile([128, 128], F32, tag="aggT")
                    nc.scalar.copy(aggT[:], ps_agg[:])
                    ps_z = psA.tile([128, HID], F32, tag="z")
                    nc.tensor.matmul(ps_z[:], aggT[:], Wl_t[l][:],
                                     start=True, stop=True)
                    z = wpool.tile([128, HID], F32, tag="zt")
                    nc.vector.tensor_tensor(z[:], ps_z[:], bsl_t[l][:], OP.add)
                    nc.sync.dma_start(z_dram[b * 128:(b + 1) * 128, :], z[:])
                    oh = wpool.tile([128, 32], F32, tag="oh")
                    nc.vector.tensor_scalar(oh[:], iota32t[:],
                                            batchloc[:, b:b + 1], None,
                                            OP.is_equal)
                    nc.tensor.matmul(psum_S1[:], oh[:], z[:],
                                     start=(b == 0), stop=(b == B - 1))
                    zsq = wpool.tile([128, HID], F32, tag="zsq")
                    nc.scalar.square(zsq[:], z[:])
                    nc.tensor.matmul(psum_S2[:], oh[:], zsq[:],
                                     start=(b == 0), stop=(b == B - 1))

                # ---- stats
                a = alphas[l]
                mu = stpool.tile([32, HID], F32, tag=f"mu{l}")
                nc.vector.tensor_scalar(mu[:], psum_S1[:], invc[:], None, OP.mult)
                musq = stpool.tile([32, HID], F32, tag=f"musq{l}")
                nc.scalar.square(musq[:], mu[:])
                t1 = stpool.tile([32, HID], F32, tag=f"t1{l}")
                nc.vector.tensor_scalar(t1[:], psum_S2[:], invc[:], None, OP.mult)
                var = stpool.tile([32, HID], F32, tag=f"var{l}")
                nc.vector.scalar_tensor_tensor(var[:], musq[:], -a * (2.0 - a),
                                               t1[:], OP.mult, OP.add)
                sd = stpool.tile([32, HID], F32, tag=f"sd{l}")
                epsc = stpool.tile([32, 1], F32, tag=f"eps{l}")
                nc.vector.memset(epsc[:], EPS)
                nc.scalar.activation(sd[:], var[:], AF.Sqrt, bias=epsc[:])
                rinv = stpool.tile([32, HID], F32, tag=f"ri{l}")
                nc.vector.reciprocal(rinv[:], sd[:])
                amus = stpool.tile([32, 2 * HID], F32, tag=f"amus{l}")
                nc.scalar.activation(amus[:, :HID], mu[:], AF.Copy, scale=float(a))
                nc.vector.tensor_tensor(amus[:, HID:], rinv[:], gml_t[l][:], OP.mult)

                # ---- apply pass
                dst = out_h if l == L - 1 else h_shard[l]
                for b in range(B):
                    z = wpool.tile([128, HID], F32, tag="zb")
                    nc.sync.dma_start(z[:], z_dram[b * 128:(b + 1) * 128, :])
                    oh = wpool.tile([128, 32], F32, tag="oh2")
                    nc.vector.tensor_scalar(oh[:], iota32t[:],
                                            batchloc[:, b:b + 1], None,
                                            OP.is_equal)
                    ps_tr = psB.tile([32, 128], F32, tag="tr")
                    nc.tensor.transpose(ps_tr[:], oh[:], ident[:])
                    ohT = wpool.tile([32, 128], F32, tag="ohT")
                    nc.vector.tensor_copy(ohT[:], ps_tr[:])
                    ps_bc = psB.tile([128, 2 * HID], F32, tag="bc")
                    nc.tensor.matmul(ps_bc[:], ohT[:], amus[:], start=True, stop=True)
                    z1 = wpool.tile([128, HID], F32, tag="z1")
                    nc.vector.tensor_tensor(z1[:], z[:], ps_bc[:, :HID], OP.subtract)
                    t2 = wpool.tile([128, HID], F32, tag="t2")
                    nc.vector.tensor_tensor(t2[:], z1[:], ps_bc[:, HID:], OP.mult)
                    o = wpool.tile([128, HID], F32, tag="o")
                    nc.vector.tensor_tensor(o[:], t2[:], btl_t[l][:], OP.add)
                    if l < L - 1:
                        nc.vector.tensor_scalar(o[:], o[:], 0.0, None, OP.max)
                    nc.sync.dma_start(dst[b * 128:(b + 1) * 128, :], o[:])

                # ---- allgather between layers
                if l < L - 1:
                    nc.gpsimd.collective_compute(
                        "AllGather", OP.bypass,
                        replica_groups=[list(range(NCORES))],
                        ins=[h_shard[l][:]],
                        outs=[h_full[l][:]],
                    )

    nc.compile()
    return nc


# ----------------------------------------------------------------------------
# public entry point
# ----------------------------------------------------------------------------

def _install_ntff_shim():
    """Optional: register the axon NTFF profiling hook so trace=True works."""
    import types, ctypes, contextlib, os
    if "antenv.axon_hooks" in sys.modules:
        return True
    so_path = "/opt/axon/libaxon_pjrt.so"
    if not os.path.exists(so_path):
        return False
    mod = types.ModuleType("antenv.axon_hooks")
    mod._hook = None
    mod.set_axon_ntff_profile_hook = lambda h: setattr(mod, "_hook", h)
    mod.get_axon_ntff_profile_hook = lambda: mod._hook
    sys.modules["antenv.axon_hooks"] = mod
    try:
        import antenv
        antenv.axon_hooks = mod
    except ImportError:
        pass
    lib = ctypes.CDLL(so_path)
    if not hasattr(lib, "axon_start_nrt_profile"):
        return False
    lib.axon_start_nrt_profile.argtypes = [ctypes.POINTER(ctypes.c_int64), ctypes.c_size_t]
    lib.axon_start_nrt_profile.restype = ctypes.c_int64
    lib.axon_stop_nrt_profile.argtypes = [ctypes.c_char_p]
    lib.axon_stop_nrt_profile.restype = ctypes.c_int64

    @contextlib.contextmanager
    def _hook(output_dir, device_ids):
        import jax
        jax.devices()
        if device_ids:
            ids = (ctypes.c_int64 * len(device_ids))(*device_ids)
            rc = lib.axon_start_nrt_profile(ids, len(device_ids))
        else:
            rc = lib.axon_start_nrt_profile(None, 0)
        if rc != 0:
            raise RuntimeError(f"axon_start_nrt_profile rc={rc}")
        try:
            yield
        finally:
            n = lib.axon_stop_nrt_profile(str(output_dir).encode())
            print(f"ntff profile: {n} file(s) -> {output_dir}", file=sys.stderr)

    mod.set_axon_ntff_profile_hook(_hook)
    return True


def kernel(x, edge_row, edge_col, edge_val, batch,
           W_in, b_in, Ws, bs, alphas, gammas, betas):
    x = np.asarray(x); batch = np.asarray(batch).astype(np.int64)
    edge_row = np.asarray(edge_row).astype(np.int64)
    edge_col = np.asarray(edge_col).astype(np.int64)
    edge_val = np.asarray(edge_val, dtype=np.float32)
    W_in = np.asarray(W_in, np.float32); b_in = np.asarray(b_in, np.float32)
    Ws = np.asarray(Ws, np.float32); bs = np.asarray(bs, np.float32)
    alphas = np.asarray(alphas, np.float32)
    gammas = np.asarray(gammas, np.float32); betas = np.asarray(betas, np.float32)
    G = 256

    in_maps, meta, perm, nodes_of = _host_prep(
        x, edge_row, edge_col, edge_val, batch,
        W_in, b_in, Ws, bs, alphas, gammas, betas, G)

    nc = _build_program(meta)
    import os
    trace = bool(os.environ.get("GCN_TRACE")) and _install_ntff_shim()
    res = run_bass_kernel_spmd(nc, in_maps, list(range(NCORES)), trace=trace)
    if trace and res.exec_time_ns is not None:
        print(f"HW exec time: {res.exec_time_ns} ns")

    N = x.shape[0]
    out = np.empty((N, HID), np.float32)
    for c in range(NCORES):
        hc = res.results[c]["out_h"]
        out[nodes_of[c]] = hc[perm[c]]
    return out



# revision 8
# speedup vs baseline: 1.7833x; 1.7833x over previous
"""ClauseGCN (3-layer GCN with GraphNorm) on 8 Trainium2 NeuronCores.

Strategy (v2, bf16 data path):
  - Nodes sharded across 8 cores by graph id (batch sorted, 256 graphs -> 32
    graphs/core).  Each core owns its nodes' rows of h and the edges whose
    DESTINATION lands on that core.
  - Per core, destination nodes are bin-packed into B blocks of 128 ("dest
    blocks").  Sparse aggregation for one block runs on the TensorEngine as
    sum_t G_t^T @ S_t accumulated in PSUM, where G_t is a [128 edge, 128 feat]
    bf16 tile of source rows and S_t[e, d] = val[e] * (dest_slot[e] == d) is
    built on the VectorEngine with one fused bf16 tensor_scalar per tile.
  - Layer 0 needs NO gather at all: node embeddings are a pure function of
    the integer inputs, so the host pre-expands val_e * [emb(combo_e), 1]
    into per-edge-slot 16-wide bf16 rows; the per-block aggregate
    agg0 = sum_t emb16_t^T @ S_t is [16, 128] and z0 = agg0^T @ WW with
    WW = [[W_in @ W_0], [b_in @ W_0]] folded on the host.
  - Layers 1/2 fetch source rows with the MoE dma_gather primitive (int16
    indices into the 8 per-core shards of the AllGather'd bf16 h).
  - The per-layer bias b_l is NOT added on the z path; it is folded into the
    GraphNorm statistics algebraically (z = z' + c with c constant over
    nodes commutes with segment means).
  - GraphNorm stats via one-hot matmuls against 32 local graph slots; the
    apply pass is out = z'*s - q with [s | q] broadcast per-node by a single
    [32]-contraction matmul (s = gamma/sd, q = (a*mu - (1-a)c)*s - beta).
  - z' stays resident in SBUF between the stats and apply passes (bf16).
  - h is AllGather'd between layers (bf16, internal DRAM, gpsimd collective).
"""

import sys
import numpy as np

sys.path.insert(0, "/opt/trn_rl_repo")

import concourse.bass as bass
import concourse.bacc as bacc
import concourse.mybir as mybir
from concourse.tile import TileContext
from concourse.bass_utils import run_bass_kernel_spmd

F32 = mybir.dt.float32
BF16 = mybir.dt.bfloat16
I16 = mybir.dt.int16
OP = mybir.AluOpType
AF = mybir.ActivationFunctionType

NCORES = 8
HID = 128
L = 3
EPS = 1e-5
SIN = 8
CH_TILES = 10                # gather-call chunk, in 128-slot tiles
NQ = 4                       # SWDGE queues
TPB = 20                     # tiles per block (sum of per-stream quotas)


# ----------------------------------------------------------------------------
# host-side prep
# ----------------------------------------------------------------------------

def _quota(b, s):
    # tiles of 128 edge slots for cell (block b, source-core s); avg 2.5
    return 2 + ((b + s) & 1)


def _pack_core(node_deg8, n_nodes, B):
    """Assign this core's nodes to B blocks of <=128 nodes s.t. the edge count
    of cell (block, src_core) stays under _quota(b, s)*128."""
    caps = np.empty((B, 8), np.int64)
    for b in range(B):
        for s in range(8):
            caps[b, s] = _quota(b, s) * 128
    loads = np.zeros((B, 8), np.int64)
    counts = np.zeros(B, np.int64)
    order = np.argsort(-node_deg8.sum(1), kind="stable")
    assign = np.full(n_nodes, -1, np.int64)
    for n in order:
        d = node_deg8[n]
        slack = caps - loads - d[None, :]
        feas = (slack.min(1) >= 0) & (counts < 128)
        if not feas.any():
            return None
        score = np.where(feas, slack.min(1), -1)
        b = int(np.argmax(score))
        assign[n] = b
        loads[b] += d
        counts[b] += 1
    return assign


def _wrap_idx16(flat):
    """MoE dma_gather index layout: [128, n/16] int16, 16-partition wrap
    replicated 8x down the partitions."""
    n = len(flat)
    blk = flat.reshape(n // 16, 16).T.astype(np.int16)   # [16, n/16]
    out = np.zeros((128, n // 16), np.int16)
    for k in range(8):
        out[16 * k:16 * (k + 1)] = blk
    return out


def _embed16(combo):
    """[n, 16] float32: [onehot6(type), log1p(arity), sin8(argpos), 1.0]."""
    cty, car, cpo = combo // 50, (combo // 10) % 5, combo % 10
    n = len(combo)
    e = np.zeros((n, 16), np.float32)
    e[np.arange(n), cty] = 1.0
    e[:, 6] = np.log1p(car.astype(np.float32))
    div_term = np.exp(np.arange(0, SIN, 2, dtype=np.float32)
                      * (-np.log(10000.0) / SIN))
    for k in range(SIN // 2):
        e[:, 7 + 2 * k] = np.sin(cpo * div_term[k])
        e[:, 8 + 2 * k] = np.cos(cpo * div_term[k])
    e[:, 15] = 1.0
    return e


def _host_prep(x, edge_row, edge_col, edge_val, batch,
               W_in, b_in, Ws, bs, alphas, gammas, betas, G):
    N = x.shape[0]
    gpc = G // NCORES                       # graphs per core

    node_core = (batch.astype(np.int64) // gpc).clip(0, NCORES - 1)
    edge_core = node_core[edge_row]         # dest core owns the edge
    src_core = node_core[edge_col]

    # ---- decide block count B (uniform across cores)
    counts_n = np.bincount(node_core, minlength=NCORES)
    counts_e = np.bincount(edge_core, minlength=NCORES)
    B = 1
    for c in range(NCORES):
        B = max(B, int(np.ceil(counts_n[c] / 128.0)),
                int(np.ceil(counts_e[c] / 2100.0)))
    B = int(np.ceil(B / 8.0) * 8)           # multiple of 8

    while True:
        Npad = B * 128
        assert Npad <= 32768, "per-core shard must fit int16 gather indices"
        # per-core packing and permutation
        perm = [None] * NCORES              # local node id -> packed row
        nodes_of = [None] * NCORES
        ok = True
        for c in range(NCORES):
            nodes_c = np.nonzero(node_core == c)[0]
            n_c = len(nodes_c)
            lid = np.full(N, -1, np.int64)
            lid[nodes_c] = np.arange(n_c)
            deg8 = np.zeros((n_c, 8), np.int64)
            m = edge_core == c
            np.add.at(deg8, (lid[edge_row[m]], src_core[m]), 1)
            assign = _pack_core(deg8, n_c, B)
            if assign is None:
                ok = False
                break
            rows = np.empty(n_c, np.int64)
            blk_count = np.zeros(B, np.int64)
            for i in np.argsort(assign, kind="stable"):
                bb = assign[i]
                rows[i] = bb * 128 + blk_count[bb]
                blk_count[bb] += 1
            perm[c] = rows
            nodes_of[c] = nodes_c
        if ok:
            break
        B += 8

    Npad = B * 128
    SL = B * 320                            # slots per stream
    CH = CH_TILES * 128
    assert SL % CH == 0

    # global packed row of every node
    grow = np.empty(N, np.int64)
    for c in range(NCORES):
        grow[nodes_of[c]] = c * Npad + perm[c]

    # combo id per node
    ty = np.clip(x[:, 0].astype(np.int64), 0, 5)
    ar = np.clip(x[:, 1].astype(np.int64), 0, 4)
    po = np.clip(x[:, 2].astype(np.int64), 0, 9)
    combo = ty * 50 + ar * 10 + po

    in_maps = []
    meta = dict(B=B, Npad=Npad, SL=SL, CH=CH, gpc=gpc)
    meta["alphas"] = [float(a) for a in alphas]

    # constant tiles (same for all cores)
    iota128 = np.tile(np.arange(128, dtype=np.float32), (128, 1))
    iota32t = np.tile(np.arange(32, dtype=np.float32), (128, 1))
    ident128 = np.eye(128, dtype=np.float32)

    # layer-0 fused input projection: z0 = (A @ [emb | 1]) @ WW  (+ c folded
    # into the norm).  WW rows 0:15 = W_in @ W_0, row 15 = b_in @ W_0.
    WW = np.zeros((16, HID), np.float32)
    WW[:15] = W_in.astype(np.float64) @ Ws[0].astype(np.float64)
    WW[15] = b_in.astype(np.float64) @ Ws[0].astype(np.float64)

    consts = dict(
        iota128=iota128.astype(np.float32), iota32t=iota32t,
        ident128=ident128, WW=WW,
    )
    for l in range(L):
        a = float(alphas[l])
        c = bs[l].astype(np.float32)               # folded bias
        k = a * (2.0 - a)
        consts[f"W_{l}"] = Ws[l].astype(np.float32)
        consts[f"gamma_rep_{l}"] = np.tile(gammas[l][None, :], (32, 1)).astype(np.float32)
        consts[f"beta_rep_{l}"] = np.tile(betas[l][None, :], (32, 1)).astype(np.float32)
        consts[f"cs_{l}"] = np.tile(((a - 1.0) * c)[None, :], (32, 1)).astype(np.float32)
        consts[f"c2_{l}"] = np.tile((2.0 * c * (1.0 - k))[None, :], (32, 1)).astype(np.float32)
        consts[f"c3_{l}"] = np.tile((c * c * (1.0 - k))[None, :], (32, 1)).astype(np.float32)

    for c in range(NCORES):
        m = edge_core == c
        er = edge_row[m]
        ec = edge_col[m]
        ev = edge_val[m].astype(np.float32)
        drow = grow[er] - c * Npad          # packed row within this core
        dblk = drow // 128
        dslot = drow % 128
        scre = node_core[ec]                # stream per edge
        sidx = grow[ec] - scre * Npad       # int16 idx into source shard
        cidx = combo[ec]

        # slot layout: per stream, block-major, quota(b,s) tiles per cell
        idx16 = np.zeros(NCORES * SL, np.int64)
        dest = np.zeros(NCORES * SL, np.float32)
        val = np.zeros(NCORES * SL, np.float32)
        csl = np.zeros(NCORES * SL, np.int64)    # combo per slot (for emb16)
        cell_off = np.zeros((B, NCORES), np.int64)
        quota_arr = np.zeros((B, NCORES), np.int64)
        for s in range(NCORES):
            off = 0
            for b in range(B):
                cell_off[b, s] = off
                quota_arr[b, s] = _quota(b, s)
                off += _quota(b, s) * 128
            assert off == SL
        key = dblk * NCORES + scre
        order = np.argsort(key, kind="stable")
        ks = key[order]
        starts = np.searchsorted(ks, np.arange(B * NCORES), side="left")
        rank = np.arange(len(order)) - starts[ks]
        bo = dblk[order]; so = scre[order]
        assert (rank < quota_arr[bo, so] * 128).all(), "cell overflow"
        flat = so * SL + cell_off[bo, so] + rank
        idx16[flat] = sidx[order]
        csl[flat] = cidx[order]
        dest[flat] = dslot[order]
        val[flat] = ev[order]
        idx16 = idx16.reshape(NCORES, SL)
        csl = csl.reshape(NCORES, SL)
        dest = dest.reshape(NCORES, SL)
        val = val.reshape(NCORES, SL)

        idx16_w = np.zeros((NCORES, 128, SL // 16), np.int16)
        for s in range(NCORES):
            idx16_w[s] = _wrap_idx16(idx16[s])

        # dest/val in [128 partitions, total-tile] col layout:
        # tile (b, s, q) at column b*TPB + cum_quota(s) + q ; partition=slot%128
        dest_cols = np.zeros((128, B * TPB), np.float32)
        val_cols = np.zeros((128, B * TPB), np.float32)
        # layer-0 per-slot val * [emb16]: [128, B*TPB*16]
        emb16 = np.zeros((128, B * TPB, 16), np.float32)
        for b in range(B):
            col = b * TPB
            for s in range(NCORES):
                qn = _quota(b, s)
                o = cell_off[b, s]
                seg_d = dest[s, o:o + qn * 128].reshape(qn, 128).T
                seg_v = val[s, o:o + qn * 128].reshape(qn, 128).T
                dest_cols[:, col:col + qn] = seg_d
                val_cols[:, col:col + qn] = seg_v
                seg_c = csl[s, o:o + qn * 128].reshape(qn, 128).T
                e16 = _embed16(seg_c.ravel()).reshape(128, qn, 16)
                emb16[:, col:col + qn, :] = e16
                col += qn

        # graphnorm per-core data
        nodes_c = nodes_of[c]
        bl = np.full(Npad, 32.0, np.float32)      # dummy rows -> slot 32
        bl[perm[c]] = (batch[nodes_c] - c * gpc).astype(np.float32)
        batchloc = bl.reshape(B, 128).T.copy()    # [128, B]
        cnt = np.zeros(32, np.float32)
        for g in range(32):
            cnt[g] = float(np.sum(batch[nodes_c] == c * gpc + g))
        invc = (1.0 / np.clip(cnt, 1.0, None)).reshape(32, 1).astype(np.float32)

        def tobf(a):
            import jax.numpy as jnp  # noqa: F401 — numpy lacks bf16; use raw uint16 trick
            return a

        im = dict(
            idx16=idx16_w.reshape(NCORES * 128, SL // 16),
            dest_cols=dest_cols, val_cols=val_cols,
            emb16=_np_bf16(emb16.reshape(128, B * TPB * 16)),
            batchloc=batchloc, invc=invc,
        )
        for kk, vv in consts.items():
            if kk == "iota128":
                im[kk] = _np_bf16(vv)
            else:
                im[kk] = vv
        in_maps.append(im)

    return in_maps, meta, perm, nodes_of


def _np_bf16(a):
    """float32 ndarray -> bfloat16 (ml_dtypes) ndarray."""
    import ml_dtypes
    return a.astype(ml_dtypes.bfloat16)


# ----------------------------------------------------------------------------
# device program
# ----------------------------------------------------------------------------

def _build_program(meta, debug=0):
    B = meta["B"]; Npad = meta["Npad"]; SL = meta["SL"]; CH = meta["CH"]
    alphas = meta["alphas"]
    n_chunks = SL // CH

    nc = bacc.Bacc(num_swdge_queues=NQ)

    # ---- IO
    t_in = {}
    for name, shape, dt in [
        ("idx16", [NCORES * 128, SL // 16], I16),
        ("dest_cols", [128, B * TPB], F32),
        ("val_cols", [128, B * TPB], F32),
        ("emb16", [128, B * TPB * 16], BF16),
        ("batchloc", [128, B], F32),
        ("invc", [32, 1], F32),
        ("iota128", [128, 128], BF16),
        ("iota32t", [128, 32], F32),
        ("ident128", [128, 128], F32),
        ("WW", [16, HID], F32),
    ]:
        t_in[name] = nc.dram_tensor(name, shape, dt, kind="ExternalInput")
    for l in range(L):
        t_in[f"W_{l}"] = nc.dram_tensor(f"W_{l}", [HID, HID], F32, kind="ExternalInput")
        for nm in ("gamma_rep", "beta_rep", "cs", "c2", "c3"):
            t_in[f"{nm}_{l}"] = nc.dram_tensor(f"{nm}_{l}", [32, HID], F32,
                                               kind="ExternalInput")

    out_h = nc.dram_tensor("out_h", [Npad, HID], F32, kind="ExternalOutput")

    h_shard = [nc.dram_tensor(f"h_shard_{l}", [Npad, HID], BF16) for l in range(L - 1)]
    h_full = [nc.dram_tensor(f"h_full_{l}", [NCORES * Npad, HID], BF16,
                             addr_space="Shared") for l in range(L - 1)]

    with TileContext(nc) as tc:
        with tc.tile_pool(name="const", bufs=1) as cpool, \
             tc.tile_pool(name="gbuf", bufs=1) as gpool, \
             tc.tile_pool(name="zres", bufs=1) as zpool, \
             tc.tile_pool(name="work", bufs=3) as wpool, \
             tc.tile_pool(name="sS", bufs=6) as spool, \
             tc.tile_pool(name="small", bufs=3) as ipool, \
             tc.tile_pool(name="stat", bufs=1) as stpool, \
             tc.tile_pool(name="psA", bufs=2, space="PSUM") as psA, \
             tc.tile_pool(name="psB", bufs=1, space="PSUM") as psB, \
             tc.tile_pool(name="psS", bufs=1, space="PSUM") as psS:

            # ---- resident constants
            def cload(name, shape, dt):
                t = cpool.tile(shape, dt, tag=name)
                nc.sync.dma_start(t[:], t_in[name][:])
                return t

            iota128 = cload("iota128", [128, 128], BF16)
            iota32t = cload("iota32t", [128, 32], F32)
            ident = cload("ident128", [128, 128], F32)
            batchloc = cload("batchloc", [128, B], F32)
            invc = cload("invc", [32, 1], F32)
            WW_t = cload("WW", [16, HID], F32)
            Wl_t = [cload(f"W_{l}", [HID, HID], F32) for l in range(1, L)]
            gm_t = [cload(f"gamma_rep_{l}", [32, HID], F32) for l in range(L)]
            bt_t = [cload(f"beta_rep_{l}", [32, HID], F32) for l in range(L)]
            cs_t = [cload(f"cs_{l}", [32, HID], F32) for l in range(L)]
            c2_t = [cload(f"c2_{l}", [32, HID], F32) for l in range(L)]
            c3_t = [cload(f"c3_{l}", [32, HID], F32) for l in range(L)]

            # ---- z residency for the apply pass (f32 for stats precision)
            z_all = zpool.tile([128, B * HID], F32, tag="z_all")

            # ---- gather buffers
            Gb = []
            for s in range(NCORES):
                row = []
                for k in range(2):
                    g = gpool.tile([128, CH], BF16, tag=f"G{s}_{k}")
                    row.append(g)
                Gb.append(row)

            # cumulative tile index per stream at block starts
            cum = np.zeros((B + 1, NCORES), np.int64)
            for b in range(B):
                for s in range(NCORES):
                    cum[b + 1, s] = cum[b, s] + _quota(b, s)

            for l in range(L):
                if l > 0:
                    hf = h_full[l - 1]
                    tables = [hf[s * Npad:(s + 1) * Npad, :] for s in range(NCORES)]

                psum_S1 = psS.tile([32, HID], F32, tag="S1")
                psum_S2 = psS.tile([32, HID], F32, tag="S2")

                emitted = [[False] * n_chunks for _ in range(NCORES)]

                def ensure_chunk(s, k):
                    if emitted[s][k]:
                        return
                    emitted[s][k] = True
                    it = ipool.tile([128, CH // 16], I16, tag="idx")
                    nc.sync.dma_start(
                        it[:], t_in["idx16"][s * 128:(s + 1) * 128,
                                             k * (CH // 16):(k + 1) * (CH // 16)])
                    g = Gb[s][k % 2]
                    nc.gpsimd.dma_gather(
                        g[:].rearrange("p (t f) -> p t f", f=HID),
                        tables[s], it[:], CH, CH, HID,
                        single_packet=False, queue_num=(s + k) % NQ,
                    )

                dvw = [None, None, None]
                for b in range(B):
                    if b % 8 == 0:
                        dw = wpool.tile([128, 8 * TPB], F32, tag="dw")
                        nc.sync.dma_start(
                            dw[:], t_in["dest_cols"][:, b * TPB:(b + 8) * TPB])
                        vw = wpool.tile([128, 8 * TPB], F32, tag="vw")
                        nc.sync.dma_start(
                            vw[:], t_in["val_cols"][:, b * TPB:(b + 8) * TPB])
                        if l == 0:
                            ew = wpool.tile([128, 8 * TPB * 16], BF16, tag="ew")
                            nc.sync.dma_start(
                                ew[:], t_in["emb16"][:, b * TPB * 16:(b + 8) * TPB * 16])
                        else:
                            ew = None
                        dvw = [dw, vw, ew]
                    dest_cols, val_cols, emb_cols = dvw
                    woff = (b % 8) * TPB

                    if l == 0:
                        ps_agg = psA.tile([16, 128], F32, tag="agg")
                    else:
                        ps_agg = psA.tile([128, 128], F32, tag="agg")
                    ti = 0
                    for s in range(NCORES):
                        qn = _quota(b, s)
                        for q in range(qn):
                            st = cum[b, s] + q          # stream tile index
                            S = spool.tile([128, 128], BF16, tag="S")
                            nc.vector.tensor_scalar(
                                S[:], iota128[:],
                                dest_cols[:, woff + ti:woff + ti + 1],
                                val_cols[:, woff + ti:woff + ti + 1],
                                OP.is_equal, OP.mult)
                            if l == 0:
                                lhs = emb_cols[:, (woff + ti) * 16:(woff + ti + 1) * 16]
                            else:
                                k = st // CH_TILES
                                col = st % CH_TILES
                                ensure_chunk(s, k)
                                G2 = Gb[s][k % 2]
                                lhs = G2[:, col * HID:(col + 1) * HID]
                            nc.tensor.matmul(
                                ps_agg[:], lhs, S[:],
                                start=(ti == 0), stop=(ti == TPB - 1))
                            ti += 1

                    if l == 0:
                        aggT = wpool.tile([16, 128], F32, tag="aggT0")
                        nc.scalar.copy(aggT[:], ps_agg[:])
                        ps_z = psA.tile([128, HID], F32, tag="z")
                        nc.tensor.matmul(ps_z[:], aggT[:], WW_t[:],
                                         start=True, stop=True)
                    else:
                        aggT = wpool.tile([128, 128], F32, tag="aggT")
                        nc.scalar.copy(aggT[:], ps_agg[:])
                        ps_z = psA.tile([128, HID], F32, tag="z")
                        nc.tensor.matmul(ps_z[:], aggT[:], Wl_t[l - 1][:],
                                         start=True, stop=True)
                    # z' (bias folded into norm stats) -> SBUF residency
                    zb = z_all[:, b * HID:(b + 1) * HID]
                    nc.scalar.copy(zb, ps_z[:])
                    oh = spool.tile([128, 32], F32, tag="oh")
                    nc.vector.tensor_scalar(oh[:], iota32t[:],
                                            batchloc[:, b:b + 1], None,
                                            OP.is_equal)
                    nc.tensor.matmul(psum_S1[:], oh[:], zb,
                                     start=(b == 0), stop=(b == B - 1))
                    zsq = wpool.tile([128, HID], F32, tag="zsq")
                    nc.scalar.square(zsq[:], zb)
                    nc.tensor.matmul(psum_S2[:], oh[:], zsq[:],
                                     start=(b == 0), stop=(b == B - 1))

                if debug == 1 + 2 * l:
                    for b in range(B):
                        o32 = wpool.tile([128, HID], F32, tag="dbg")
                        nc.vector.tensor_copy(o32[:], z_all[:, b * HID:(b + 1) * HID])
                        nc.sync.dma_start(out_h[b * 128:(b + 1) * 128, :], o32[:])
                    break
                # ---- stats  (z = z' + c, c folded per-feature constant)
                a = alphas[l]
                k_ = a * (2.0 - a)
                mu = stpool.tile([32, HID], F32, tag=f"mu{l}")
                nc.vector.tensor_scalar(mu[:], psum_S1[:], invc[:], None, OP.mult)
                t1 = stpool.tile([32, HID], F32, tag=f"t1{l}")
                nc.vector.tensor_scalar(t1[:], psum_S2[:], invc[:], None, OP.mult)
                # amu_c = a*mu' + (a-1)*c
                amuc = stpool.tile([32, HID], F32, tag=f"amuc{l}")
                nc.vector.scalar_tensor_tensor(amuc[:], mu[:], a, cs_t[l][:],
                                               OP.mult, OP.add)
                # var = t1 - k*mu'^2 + mu'*c2 + c3
                musq = stpool.tile([32, HID], F32, tag=f"musq{l}")
                nc.scalar.square(musq[:], mu[:])
                var = stpool.tile([32, HID], F32, tag=f"var{l}")
                nc.vector.scalar_tensor_tensor(var[:], musq[:], -k_, t1[:],
                                               OP.mult, OP.add)
                w2 = stpool.tile([32, HID], F32, tag=f"w2{l}")
                nc.vector.tensor_tensor(w2[:], mu[:], c2_t[l][:], OP.mult)
                nc.vector.tensor_tensor(var[:], var[:], w2[:], OP.add)
                nc.vector.tensor_tensor(var[:], var[:], c3_t[l][:], OP.add)
                sd = stpool.tile([32, HID], F32, tag=f"sd{l}")
                epsc = stpool.tile([32, 1], F32, tag=f"eps{l}")
                nc.vector.memset(epsc[:], EPS)
                nc.scalar.activation(sd[:], var[:], AF.Sqrt, bias=epsc[:])
                rinv = stpool.tile([32, HID], F32, tag=f"ri{l}")
                nc.vector.reciprocal(rinv[:], sd[:])
                # amus = [s | q]  with s = rinv*gamma, q = amu_c*s - beta
                ssc = stpool.tile([32, HID], F32, tag=f"ssc{l}")
                nc.vector.tensor_tensor(ssc[:], rinv[:], gm_t[l][:], OP.mult)
                amus = stpool.tile([32, 2 * HID], F32, tag=f"amus{l}")
                nc.vector.tensor_copy(amus[:, :HID], ssc[:])
                qt = stpool.tile([32, HID], F32, tag=f"qt{l}")
                nc.vector.tensor_tensor(qt[:], amuc[:], ssc[:], OP.mult)
                nc.vector.tensor_tensor(amus[:, HID:], qt[:], bt_t[l][:],
                                        OP.subtract)

                # ---- apply pass: out = z'*s - q  (+relu except last layer)
                dbg_apply = (debug == 2 + 2 * l)
                dst = out_h if (l == L - 1 or dbg_apply) else h_shard[l]
                for b in range(B):
                    oh2 = spool.tile([128, 32], F32, tag="oh2")
                    nc.vector.tensor_scalar(oh2[:], iota32t[:],
                                            batchloc[:, b:b + 1], None,
                                            OP.is_equal)
                    ps_tr = psB.tile([32, 128], F32, tag="tr")
                    nc.tensor.transpose(ps_tr[:], oh2[:], ident[:])
                    ohT = wpool.tile([32, 128], F32, tag="ohT")
                    nc.vector.tensor_copy(ohT[:], ps_tr[:])
                    ps_bc = psB.tile([128, 2 * HID], F32, tag="bc")
                    nc.tensor.matmul(ps_bc[:], ohT[:], amus[:], start=True, stop=True)
                    bc = wpool.tile([128, 2 * HID], F32, tag="bcs")
                    nc.scalar.copy(bc[:], ps_bc[:])
                    zb = z_all[:, b * HID:(b + 1) * HID]
                    t2 = wpool.tile([128, HID], F32, tag="t2f")
                    nc.vector.tensor_tensor(t2[:], zb, bc[:, :HID], OP.mult)
                    if l < L - 1 and not dbg_apply:
                        o = wpool.tile([128, HID], BF16, tag="o")
                        nc.vector.tensor_tensor(o[:], t2[:], bc[:, HID:], OP.subtract)
                        nc.vector.tensor_scalar(o[:], o[:], 0.0, None, OP.max)
                    else:
                        o = wpool.tile([128, HID], F32, tag="of")
                        nc.vector.tensor_tensor(o[:], t2[:], bc[:, HID:], OP.subtract)
                        if dbg_apply and l < L - 1:
                            nc.vector.tensor_scalar(o[:], o[:], 0.0, None, OP.max)
                    nc.sync.dma_start(dst[b * 128:(b + 1) * 128, :], o[:])

                if debug == 2 + 2 * l:
                    break
                # ---- allgather between layers
                if l < L - 1:
                    nc.gpsimd.collective_compute(
                        "AllGather", OP.bypass,
                        replica_groups=[list(range(NCORES))],
                        ins=[h_shard[l][:]],
                        outs=[h_full[l][:]],
                    )

    nc.compile()
    return nc


# ----------------------------------------------------------------------------
# public entry point
# ----------------------------------------------------------------------------

def _install_ntff_shim():
    """Optional: register the axon NTFF profiling hook so trace=True works."""
    import types, ctypes, contextlib, os
    if "antenv.axon_hooks" in sys.modules:
        return True
    so_path = "/opt/axon/libaxon_pjrt.so"
    if not os.path.exists(so_path):
        return False
    mod = types.ModuleType("antenv.axon_hooks")
    mod._hook = None
    mod.set_axon_ntff_profile_hook = lambda h: setattr(mod, "_hook", h)
    mod.get_axon_ntff_profile_hook = lambda: mod._hook
    sys.modules["antenv.axon_hooks"] = mod
    try:
        import antenv
        antenv.axon_hooks = mod
    except ImportError:
        pass
    lib = ctypes.CDLL(so_path)
    if not hasattr(lib, "axon_start_nrt_profile"):
        return False
    lib.axon_start_nrt_profile.argtypes = [ctypes.POINTER(ctypes.c_int64), ctypes.c_size_t]
    lib.axon_start_nrt_profile.restype = ctypes.c_int64
    lib.axon_stop_nrt_profile.argtypes = [ctypes.c_char_p]
    lib.axon_stop_nrt_profile.restype = ctypes.c_int64

    @contextlib.contextmanager
    def _hook(output_dir, device_ids):
        import jax
        jax.devices()
        if device_ids:
            ids = (ctypes.c_int64 * len(device_ids))(*device_ids)
            rc = lib.axon_start_nrt_profile(ids, len(device_ids))
        else:
            rc = lib.axon_start_nrt_profile(None, 0)
        if rc != 0:
            raise RuntimeError(f"axon_start_nrt_profile rc={rc}")
        try:
            yield
        finally:
            n = lib.axon_stop_nrt_profile(str(output_dir).encode())
            print(f"ntff profile: {n} file(s) -> {output_dir}", file=sys.stderr)

    mod.set_axon_ntff_profile_hook(_hook)
    return True


def kernel(x, edge_row, edge_col, edge_val, batch,
           W_in, b_in, Ws, bs, alphas, gammas, betas):
    x = np.asarray(x); batch = np.asarray(batch).astype(np.int64)
    edge_row = np.asarray(edge_row).astype(np.int64)
    edge_col = np.asarray(edge_col).astype(np.int64)
    edge_val = np.asarray(edge_val, dtype=np.float32)
    W_in = np.asarray(W_in, np.float32); b_in = np.asarray(b_in, np.float32)
    Ws = np.asarray(Ws, np.float32); bs = np.asarray(bs, np.float32)
    alphas = np.asarray(alphas, np.float32)
    gammas = np.asarray(gammas, np.float32); betas = np.asarray(betas, np.float32)
    G = 256

    in_maps, meta, perm, nodes_of = _host_prep(
        x, edge_row, edge_col, edge_val, batch,
        W_in, b_in, Ws, bs, alphas, gammas, betas, G)

    import os
    debug = int(os.environ.get("GCN_DEBUG", "0"))
    nc = _build_program(meta, debug=debug)
    trace = bool(os.environ.get("GCN_TRACE")) and _install_ntff_shim()
    res = run_bass_kernel_spmd(nc, in_maps, list(range(NCORES)), trace=trace)
    if trace and res.exec_time_ns is not None:
        print(f"HW exec time: {res.exec_time_ns} ns")

    N = x.shape[0]
    out = np.empty((N, HID), np.float32)
    for c in range(NCORES):
        hc = res.results[c]["out_h"]
        out[nodes_of[c]] = hc[perm[c]]
    return out


# revision 10
# speedup vs baseline: 2.3954x; 1.3432x over previous
"""ClauseGCN (3-layer GCN with GraphNorm) on 8 Trainium2 NeuronCores.

Strategy (v2, bf16 data path):
  - Nodes sharded across 8 cores by graph id (batch sorted, 256 graphs -> 32
    graphs/core).  Each core owns its nodes' rows of h and the edges whose
    DESTINATION lands on that core.
  - Per core, destination nodes are bin-packed into B blocks of 128 ("dest
    blocks").  Sparse aggregation for one block runs on the TensorEngine as
    sum_t G_t^T @ S_t accumulated in PSUM, where G_t is a [128 edge, 128 feat]
    bf16 tile of source rows and S_t[e, d] = val[e] * (dest_slot[e] == d) is
    built on the VectorEngine with one fused bf16 tensor_scalar per tile.
  - Layer 0 needs NO gather at all: node embeddings are a pure function of
    the integer inputs, so the host pre-expands val_e * [emb(combo_e), 1]
    into per-edge-slot 16-wide bf16 rows; the per-block aggregate
    agg0 = sum_t emb16_t^T @ S_t is [16, 128] and z0 = agg0^T @ WW with
    WW = [[W_in @ W_0], [b_in @ W_0]] folded on the host.
  - Layers 1/2 fetch source rows with the MoE dma_gather primitive (int16
    indices into the 8 per-core shards of the AllGather'd bf16 h).
  - The per-layer bias b_l is NOT added on the z path; it is folded into the
    GraphNorm statistics algebraically (z = z' + c with c constant over
    nodes commutes with segment means).
  - GraphNorm stats via one-hot matmuls against 32 local graph slots; the
    apply pass is out = z'*s - q with [s | q] broadcast per-node by a single
    [32]-contraction matmul (s = gamma/sd, q = (a*mu - (1-a)c)*s - beta).
  - z' stays resident in SBUF between the stats and apply passes (bf16).
  - h is AllGather'd between layers (bf16, internal DRAM, gpsimd collective).
"""

import sys
import numpy as np

sys.path.insert(0, "/opt/trn_rl_repo")

import concourse.bass as bass
import concourse.bacc as bacc
import concourse.mybir as mybir
from concourse.tile import TileContext
from concourse.bass_utils import run_bass_kernel_spmd

F32 = mybir.dt.float32
BF16 = mybir.dt.bfloat16
I16 = mybir.dt.int16
OP = mybir.AluOpType
AF = mybir.ActivationFunctionType

NCORES = 8
HID = 128
L = 3
EPS = 1e-5
SIN = 8
CH_TILES = 13                # gather-call chunk, in 128-slot tiles
NQ = 4                       # SWDGE queues
TPB = 17                     # tiles per block (sum of per-stream quotas)


# ----------------------------------------------------------------------------
# host-side prep
# ----------------------------------------------------------------------------

def _quota(b, s):
    # tiles of 128 edge slots for cell (block b, source-core s); avg 2.125
    return 2 + (1 if ((b + s) & 7) == 0 else 0)


def _pack_core(node_deg8, n_nodes, B):
    """Assign this core's nodes to B blocks of <=128 nodes s.t. the edge count
    of cell (block, src_core) stays under _quota(b, s)*128."""
    caps = np.empty((B, 8), np.int64)
    for b in range(B):
        for s in range(8):
            caps[b, s] = _quota(b, s) * 128
    loads = np.zeros((B, 8), np.int64)
    counts = np.zeros(B, np.int64)
    order = np.argsort(-node_deg8.sum(1), kind="stable")
    assign = np.full(n_nodes, -1, np.int64)
    for n in order:
        d = node_deg8[n]
        slack = caps - loads - d[None, :]
        feas = (slack.min(1) >= 0) & (counts < 128)
        if not feas.any():
            return None
        score = np.where(feas, slack.min(1), -1)
        b = int(np.argmax(score))
        assign[n] = b
        loads[b] += d
        counts[b] += 1
    return assign


def _wrap_idx16(flat):
    """MoE dma_gather index layout: [128, n/16] int16, 16-partition wrap
    replicated 8x down the partitions."""
    n = len(flat)
    blk = flat.reshape(n // 16, 16).T.astype(np.int16)   # [16, n/16]
    out = np.zeros((128, n // 16), np.int16)
    for k in range(8):
        out[16 * k:16 * (k + 1)] = blk
    return out


def _embed16(combo):
    """[n, 16] float32: [onehot6(type), log1p(arity), sin8(argpos), 1.0]."""
    cty, car, cpo = combo // 50, (combo // 10) % 5, combo % 10
    n = len(combo)
    e = np.zeros((n, 16), np.float32)
    e[np.arange(n), cty] = 1.0
    e[:, 6] = np.log1p(car.astype(np.float32))
    div_term = np.exp(np.arange(0, SIN, 2, dtype=np.float32)
                      * (-np.log(10000.0) / SIN))
    for k in range(SIN // 2):
        e[:, 7 + 2 * k] = np.sin(cpo * div_term[k])
        e[:, 8 + 2 * k] = np.cos(cpo * div_term[k])
    e[:, 15] = 1.0
    return e


def _host_prep(x, edge_row, edge_col, edge_val, batch,
               W_in, b_in, Ws, bs, alphas, gammas, betas, G):
    N = x.shape[0]
    gpc = G // NCORES                       # graphs per core

    node_core = (batch.astype(np.int64) // gpc).clip(0, NCORES - 1)
    edge_core = node_core[edge_row]         # dest core owns the edge
    src_core = node_core[edge_col]

    # ---- decide block count B (uniform across cores)
    counts_n = np.bincount(node_core, minlength=NCORES)
    counts_e = np.bincount(edge_core, minlength=NCORES)
    B = 1
    for c in range(NCORES):
        B = max(B, int(np.ceil(counts_n[c] / 128.0)),
                int(np.ceil(counts_e[c] / 2000.0)))
    B = int(np.ceil(B / 8.0) * 8)           # multiple of 8

    while True:
        Npad = B * 128
        assert Npad <= 32768, "per-core shard must fit int16 gather indices"
        # per-core packing and permutation
        perm = [None] * NCORES              # local node id -> packed row
        nodes_of = [None] * NCORES
        ok = True
        for c in range(NCORES):
            nodes_c = np.nonzero(node_core == c)[0]
            n_c = len(nodes_c)
            lid = np.full(N, -1, np.int64)
            lid[nodes_c] = np.arange(n_c)
            deg8 = np.zeros((n_c, 8), np.int64)
            m = edge_core == c
            np.add.at(deg8, (lid[edge_row[m]], src_core[m]), 1)
            assign = _pack_core(deg8, n_c, B)
            if assign is None:
                ok = False
                break
            rows = np.empty(n_c, np.int64)
            blk_count = np.zeros(B, np.int64)
            for i in np.argsort(assign, kind="stable"):
                bb = assign[i]
                rows[i] = bb * 128 + blk_count[bb]
                blk_count[bb] += 1
            perm[c] = rows
            nodes_of[c] = nodes_c
        if ok:
            break
        B += 8

    Npad = B * 128
    SL = B * 272                            # slots per stream (17*B/8 tiles)
    CH = CH_TILES * 128

    # global packed row of every node
    grow = np.empty(N, np.int64)
    for c in range(NCORES):
        grow[nodes_of[c]] = c * Npad + perm[c]

    # combo id per node
    ty = np.clip(x[:, 0].astype(np.int64), 0, 5)
    ar = np.clip(x[:, 1].astype(np.int64), 0, 4)
    po = np.clip(x[:, 2].astype(np.int64), 0, 9)
    combo = ty * 50 + ar * 10 + po

    in_maps = []
    meta = dict(B=B, Npad=Npad, SL=SL, CH=CH, gpc=gpc)
    meta["alphas"] = [float(a) for a in alphas]

    # constant tiles (same for all cores)
    iota128 = np.tile(np.arange(128, dtype=np.float32), (128, 1))
    iota32t = np.tile(np.arange(32, dtype=np.float32), (128, 1))
    ident128 = np.eye(128, dtype=np.float32)

    # layer-0 fused input projection: z0 = (A @ [emb | 1]) @ WW  (+ c folded
    # into the norm).  WW rows 0:15 = W_in @ W_0, row 15 = b_in @ W_0.
    WW = np.zeros((16, HID), np.float32)
    WW[:15] = W_in.astype(np.float64) @ Ws[0].astype(np.float64)
    WW[15] = b_in.astype(np.float64) @ Ws[0].astype(np.float64)

    consts = dict(
        iota128=iota128.astype(np.float32), iota32t=iota32t,
        ident128=ident128, WW=WW,
    )
    for l in range(L):
        a = float(alphas[l])
        c = bs[l].astype(np.float32)               # folded bias
        k = a * (2.0 - a)
        consts[f"W_{l}"] = Ws[l].astype(np.float32)
        consts[f"gamma_rep_{l}"] = np.tile(gammas[l][None, :], (32, 1)).astype(np.float32)
        consts[f"beta_rep_{l}"] = np.tile(betas[l][None, :], (32, 1)).astype(np.float32)
        consts[f"cs_{l}"] = np.tile(((a - 1.0) * c)[None, :], (32, 1)).astype(np.float32)
        consts[f"c2_{l}"] = np.tile((2.0 * c * (1.0 - k))[None, :], (32, 1)).astype(np.float32)
        consts[f"c3_{l}"] = np.tile((c * c * (1.0 - k))[None, :], (32, 1)).astype(np.float32)

    for c in range(NCORES):
        m = edge_core == c
        er = edge_row[m]
        ec = edge_col[m]
        ev = edge_val[m].astype(np.float32)
        drow = grow[er] - c * Npad          # packed row within this core
        dblk = drow // 128
        dslot = drow % 128
        scre = node_core[ec]                # stream per edge
        sidx = grow[ec] - scre * Npad       # int16 idx into source shard
        cidx = combo[ec]

        # slot layout: per stream, block-major, quota(b,s) tiles per cell
        idx16 = np.zeros(NCORES * SL, np.int64)
        dest = np.zeros(NCORES * SL, np.float32)
        val = np.zeros(NCORES * SL, np.float32)
        csl = np.zeros(NCORES * SL, np.int64)    # combo per slot (for emb16)
        cell_off = np.zeros((B, NCORES), np.int64)
        quota_arr = np.zeros((B, NCORES), np.int64)
        for s in range(NCORES):
            off = 0
            for b in range(B):
                cell_off[b, s] = off
                quota_arr[b, s] = _quota(b, s)
                off += _quota(b, s) * 128
            assert off == SL
        key = dblk * NCORES + scre
        order = np.argsort(key, kind="stable")
        ks = key[order]
        starts = np.searchsorted(ks, np.arange(B * NCORES), side="left")
        rank = np.arange(len(order)) - starts[ks]
        bo = dblk[order]; so = scre[order]
        assert (rank < quota_arr[bo, so] * 128).all(), "cell overflow"
        flat = so * SL + cell_off[bo, so] + rank
        idx16[flat] = sidx[order]
        csl[flat] = cidx[order]
        dest[flat] = dslot[order]
        val[flat] = ev[order]
        idx16 = idx16.reshape(NCORES, SL)
        csl = csl.reshape(NCORES, SL)
        dest = dest.reshape(NCORES, SL)
        val = val.reshape(NCORES, SL)

        idx16_w = np.zeros((NCORES, 128, SL // 16), np.int16)
        for s in range(NCORES):
            idx16_w[s] = _wrap_idx16(idx16[s])

        # dest/val in [128 partitions, total-tile] col layout:
        # tile (b, s, q) at column b*TPB + cum_quota(s) + q ; partition=slot%128
        dest_cols = np.zeros((128, B * TPB), np.float32)
        val_cols = np.zeros((128, B * TPB), np.float32)
        # layer-0 per-slot val * [emb16]: [128, B*TPB*16]
        emb16 = np.zeros((128, B * TPB, 16), np.float32)
        for b in range(B):
            col = b * TPB
            for s in range(NCORES):
                qn = _quota(b, s)
                o = cell_off[b, s]
                seg_d = dest[s, o:o + qn * 128].reshape(qn, 128).T
                seg_v = val[s, o:o + qn * 128].reshape(qn, 128).T
                dest_cols[:, col:col + qn] = seg_d
                val_cols[:, col:col + qn] = seg_v
                seg_c = csl[s, o:o + qn * 128].reshape(qn, 128).T
                e16 = _embed16(seg_c.ravel()).reshape(128, qn, 16)
                emb16[:, col:col + qn, :] = e16
                col += qn

        # graphnorm per-core data
        nodes_c = nodes_of[c]
        bl = np.full(Npad, 32.0, np.float32)      # dummy rows -> slot 32
        bl[perm[c]] = (batch[nodes_c] - c * gpc).astype(np.float32)
        batchloc = bl.reshape(B, 128).T.copy()    # [128, B]
        cnt = np.zeros(32, np.float32)
        for g in range(32):
            cnt[g] = float(np.sum(batch[nodes_c] == c * gpc + g))
        invc = (1.0 / np.clip(cnt, 1.0, None)).reshape(32, 1).astype(np.float32)

        def tobf(a):
            import jax.numpy as jnp  # noqa: F401 — numpy lacks bf16; use raw uint16 trick
            return a

        im = dict(
            idx16=idx16_w.reshape(NCORES * 128, SL // 16),
            dest_cols=dest_cols, val_cols=val_cols,
            emb16=_np_bf16(emb16.reshape(128, B * TPB * 16)),
            batchloc=batchloc, invc=invc,
        )
        for kk, vv in consts.items():
            if kk == "iota128":
                im[kk] = _np_bf16(vv)
            else:
                im[kk] = vv
        in_maps.append(im)

    return in_maps, meta, perm, nodes_of


def _np_bf16(a):
    """float32 ndarray -> bfloat16 (ml_dtypes) ndarray."""
    import ml_dtypes
    return a.astype(ml_dtypes.bfloat16)


# ----------------------------------------------------------------------------
# device program
# ----------------------------------------------------------------------------

def _build_program(meta, debug=0):
    B = meta["B"]; Npad = meta["Npad"]; SL = meta["SL"]; CH = meta["CH"]
    alphas = meta["alphas"]
    t_per_stream = SL // 128
    n_chunks = (t_per_stream + CH_TILES - 1) // CH_TILES

    nc = bacc.Bacc(num_swdge_queues=NQ)

    # ---- IO
    t_in = {}
    for name, shape, dt in [
        ("idx16", [NCORES * 128, SL // 16], I16),
        ("dest_cols", [128, B * TPB], F32),
        ("val_cols", [128, B * TPB], F32),
        ("emb16", [128, B * TPB * 16], BF16),
        ("batchloc", [128, B], F32),
        ("invc", [32, 1], F32),
        ("iota128", [128, 128], BF16),
        ("iota32t", [128, 32], F32),
        ("ident128", [128, 128], F32),
        ("WW", [16, HID], F32),
    ]:
        t_in[name] = nc.dram_tensor(name, shape, dt, kind="ExternalInput")
    for l in range(L):
        t_in[f"W_{l}"] = nc.dram_tensor(f"W_{l}", [HID, HID], F32, kind="ExternalInput")
        for nm in ("gamma_rep", "beta_rep", "cs", "c2", "c3"):
            t_in[f"{nm}_{l}"] = nc.dram_tensor(f"{nm}_{l}", [32, HID], F32,
                                               kind="ExternalInput")

    out_h = nc.dram_tensor("out_h", [Npad, HID], F32, kind="ExternalOutput")

    h_shard = [nc.dram_tensor(f"h_shard_{l}", [Npad, HID], BF16) for l in range(L - 1)]
    h_full = [nc.dram_tensor(f"h_full_{l}", [NCORES * Npad, HID], BF16,
                             addr_space="Shared") for l in range(L - 1)]

    with TileContext(nc) as tc:
        with tc.tile_pool(name="const", bufs=1) as cpool, \
             tc.tile_pool(name="gbuf", bufs=1) as gpool, \
             tc.tile_pool(name="zres", bufs=1) as zpool, \
             tc.tile_pool(name="work", bufs=3) as wpool, \
             tc.tile_pool(name="sS", bufs=10) as spool, \
             tc.tile_pool(name="small", bufs=3) as ipool, \
             tc.tile_pool(name="stat", bufs=1) as stpool, \
             tc.tile_pool(name="psA", bufs=2, space="PSUM") as psA, \
             tc.tile_pool(name="psS", bufs=1, space="PSUM") as psS:

            # ---- resident constants
            def cload(name, shape, dt):
                t = cpool.tile(shape, dt, tag=name)
                nc.sync.dma_start(t[:], t_in[name][:])
                return t

            iota128 = cload("iota128", [128, 128], BF16)
            iota32t = cload("iota32t", [128, 32], F32)
            ident = cload("ident128", [128, 128], F32)
            batchloc = cload("batchloc", [128, B], F32)
            invc = cload("invc", [32, 1], F32)
            WW_t = cload("WW", [16, HID], F32)
            Wl_t = [cload(f"W_{l}", [HID, HID], F32) for l in range(1, L)]
            gm_t = [cload(f"gamma_rep_{l}", [32, HID], F32) for l in range(L)]
            bt_t = [cload(f"beta_rep_{l}", [32, HID], F32) for l in range(L)]
            cs_t = [cload(f"cs_{l}", [32, HID], F32) for l in range(L)]
            c2_t = [cload(f"c2_{l}", [32, HID], F32) for l in range(L)]
            c3_t = [cload(f"c3_{l}", [32, HID], F32) for l in range(L)]

            # ---- z residency for the apply pass (f32 for stats precision)
            z_all = zpool.tile([128, B * HID], F32, tag="z_all")

            # ---- gather buffers
            Gb = []
            for s in range(NCORES):
                row = []
                for k in range(2):
                    g = gpool.tile([128, CH], BF16, tag=f"G{s}_{k}")
                    row.append(g)
                Gb.append(row)

            # cumulative tile index per stream at block starts
            cum = np.zeros((B + 1, NCORES), np.int64)
            for b in range(B):
                for s in range(NCORES):
                    cum[b + 1, s] = cum[b, s] + _quota(b, s)

            for l in range(L):
                if l > 0:
                    hf = h_full[l - 1]
                    tables = [hf[s * Npad:(s + 1) * Npad, :] for s in range(NCORES)]

                psum_S1 = psS.tile([32, HID], F32, tag="S1")
                psum_S2 = psS.tile([32, HID], F32, tag="S2")

                emitted = [[False] * n_chunks for _ in range(NCORES)]

                def ensure_chunk(s, k):
                    if emitted[s][k]:
                        return
                    emitted[s][k] = True
                    lo = k * CH_TILES
                    hi = min(lo + CH_TILES, t_per_stream)
                    nidx = (hi - lo) * 128
                    it = ipool.tile([128, CH // 16], I16, tag="idx")
                    nc.sync.dma_start(
                        it[:, :nidx // 16],
                        t_in["idx16"][s * 128:(s + 1) * 128,
                                      lo * 8:lo * 8 + nidx // 16])
                    g = Gb[s][k % 2]
                    nc.gpsimd.dma_gather(
                        g[:, :nidx].rearrange("p (t f) -> p t f", f=HID),
                        tables[s], it[:, :nidx // 16], nidx, nidx, HID,
                        single_packet=False, queue_num=(s + k) % NQ,
                    )

                dvw = [None, None, None]
                for b in range(B):
                    if b % 8 == 0:
                        dw = wpool.tile([128, 8 * TPB], F32, tag="dw")
                        nc.sync.dma_start(
                            dw[:], t_in["dest_cols"][:, b * TPB:(b + 8) * TPB])
                        vw = wpool.tile([128, 8 * TPB], F32, tag="vw")
                        nc.sync.dma_start(
                            vw[:], t_in["val_cols"][:, b * TPB:(b + 8) * TPB])
                        if l == 0:
                            ew = wpool.tile([128, 8 * TPB * 16], BF16, tag="ew")
                            nc.sync.dma_start(
                                ew[:], t_in["emb16"][:, b * TPB * 16:(b + 8) * TPB * 16])
                        else:
                            ew = None
                        dvw = [dw, vw, ew]
                    dest_cols, val_cols, emb_cols = dvw
                    woff = (b % 8) * TPB

                    if l == 0:
                        ps_agg = psA.tile([16, 128], F32, tag="agg")
                    else:
                        ps_agg = psA.tile([128, 128], F32, tag="agg")
                    ti = 0
                    for s in range(NCORES):
                        qn = _quota(b, s)
                        for q in range(qn):
                            st = cum[b, s] + q          # stream tile index
                            S = spool.tile([128, 128], BF16, tag="S")
                            nc.vector.tensor_scalar(
                                S[:], iota128[:],
                                dest_cols[:, woff + ti:woff + ti + 1],
                                val_cols[:, woff + ti:woff + ti + 1],
                                OP.is_equal, OP.mult)
                            if l == 0:
                                lhs = emb_cols[:, (woff + ti) * 16:(woff + ti + 1) * 16]
                            else:
                                k = st // CH_TILES
                                col = st % CH_TILES
                                ensure_chunk(s, k)
                                G2 = Gb[s][k % 2]
                                lhs = G2[:, col * HID:(col + 1) * HID]
                            nc.tensor.matmul(
                                ps_agg[:], lhs, S[:],
                                start=(ti == 0), stop=(ti == TPB - 1))
                            ti += 1

                    if l == 0:
                        aggT = wpool.tile([16, 128], F32, tag="aggT0")
                        nc.scalar.copy(aggT[:], ps_agg[:])
                        ps_z = psA.tile([128, HID], F32, tag="z")
                        nc.tensor.matmul(ps_z[:], aggT[:], WW_t[:],
                                         start=True, stop=True)
                    else:
                        aggT = wpool.tile([128, 128], F32, tag="aggT")
                        nc.scalar.copy(aggT[:], ps_agg[:])
                        ps_z = psA.tile([128, HID], F32, tag="z")
                        nc.tensor.matmul(ps_z[:], aggT[:], Wl_t[l - 1][:],
                                         start=True, stop=True)
                    # z' (bias folded into norm stats) -> SBUF residency
                    zb = z_all[:, b * HID:(b + 1) * HID]
                    nc.scalar.copy(zb, ps_z[:])
                    oh = spool.tile([128, 32], F32, tag="oh")
                    nc.vector.tensor_scalar(oh[:], iota32t[:],
                                            batchloc[:, b:b + 1], None,
                                            OP.is_equal)
                    nc.tensor.matmul(psum_S1[:], oh[:], zb,
                                     start=(b == 0), stop=(b == B - 1))
                    zsq = wpool.tile([128, HID], F32, tag="zsq")
                    nc.scalar.square(zsq[:], zb)
                    nc.tensor.matmul(psum_S2[:], oh[:], zsq[:],
                                     start=(b == 0), stop=(b == B - 1))

                if debug == 1 + 2 * l:
                    for b in range(B):
                        o32 = wpool.tile([128, HID], F32, tag="dbg")
                        nc.vector.tensor_copy(o32[:], z_all[:, b * HID:(b + 1) * HID])
                        nc.sync.dma_start(out_h[b * 128:(b + 1) * 128, :], o32[:])
                    break
                # ---- stats  (z = z' + c, c folded per-feature constant)
                a = alphas[l]
                k_ = a * (2.0 - a)
                mu = stpool.tile([32, HID], F32, tag="mu")
                nc.vector.tensor_scalar(mu[:], psum_S1[:], invc[:], None, OP.mult)
                t1 = stpool.tile([32, HID], F32, tag="t1")
                nc.vector.tensor_scalar(t1[:], psum_S2[:], invc[:], None, OP.mult)
                # amu_c = a*mu' + (a-1)*c
                amuc = stpool.tile([32, HID], F32, tag="amuc")
                nc.vector.scalar_tensor_tensor(amuc[:], mu[:], a, cs_t[l][:],
                                               OP.mult, OP.add)
                # var = t1 - k*mu'^2 + mu'*c2 + c3
                musq = stpool.tile([32, HID], F32, tag="musq")
                nc.scalar.square(musq[:], mu[:])
                var = stpool.tile([32, HID], F32, tag="var")
                nc.vector.scalar_tensor_tensor(var[:], musq[:], -k_, t1[:],
                                               OP.mult, OP.add)
                w2 = stpool.tile([32, HID], F32, tag="w2")
                nc.vector.tensor_tensor(w2[:], mu[:], c2_t[l][:], OP.mult)
                nc.vector.tensor_tensor(var[:], var[:], w2[:], OP.add)
                nc.vector.tensor_tensor(var[:], var[:], c3_t[l][:], OP.add)
                sd = stpool.tile([32, HID], F32, tag="sd")
                epsc = stpool.tile([32, 1], F32, tag="eps")
                nc.vector.memset(epsc[:], EPS)
                nc.scalar.activation(sd[:], var[:], AF.Sqrt, bias=epsc[:])
                rinv = stpool.tile([32, HID], F32, tag="ri")
                nc.vector.reciprocal(rinv[:], sd[:])
                # amus = [s | q]  with s = rinv*gamma, q = amu_c*s - beta
                ssc = stpool.tile([32, HID], F32, tag="ssc")
                nc.vector.tensor_tensor(ssc[:], rinv[:], gm_t[l][:], OP.mult)
                amus = stpool.tile([32, 2 * HID], F32, tag="amus")
                nc.vector.tensor_copy(amus[:, :HID], ssc[:])
                qt = stpool.tile([32, HID], F32, tag="qt")
                nc.vector.tensor_tensor(qt[:], amuc[:], ssc[:], OP.mult)
                nc.vector.tensor_tensor(amus[:, HID:], qt[:], bt_t[l][:],
                                        OP.subtract)

                # ---- apply pass: out = z'*s - q  (+relu except last layer)
                dbg_apply = (debug == 2 + 2 * l)
                dst = out_h if (l == L - 1 or dbg_apply) else h_shard[l]
                for b in range(B):
                    oh2 = spool.tile([128, 32], F32, tag="oh2")
                    nc.vector.tensor_scalar(oh2[:], iota32t[:],
                                            batchloc[:, b:b + 1], None,
                                            OP.is_equal)
                    ps_tr = psA.tile([32, 128], F32, tag="agg")
                    nc.tensor.transpose(ps_tr[:], oh2[:], ident[:])
                    ohT = wpool.tile([32, 128], F32, tag="ohT")
                    nc.vector.tensor_copy(ohT[:], ps_tr[:])
                    ps_bc = psA.tile([128, 2 * HID], F32, tag="z")
                    nc.tensor.matmul(ps_bc[:], ohT[:], amus[:], start=True, stop=True)
                    bc = wpool.tile([128, 2 * HID], F32, tag="bcs")
                    nc.scalar.copy(bc[:], ps_bc[:])
                    zb = z_all[:, b * HID:(b + 1) * HID]
                    t2 = wpool.tile([128, HID], F32, tag="t2f")
                    nc.vector.tensor_tensor(t2[:], zb, bc[:, :HID], OP.mult)
                    if l < L - 1 and not dbg_apply:
                        o = wpool.tile([128, HID], BF16, tag="o")
                        nc.vector.tensor_tensor(o[:], t2[:], bc[:, HID:], OP.subtract)
                        nc.vector.tensor_scalar(o[:], o[:], 0.0, None, OP.max)
                    else:
                        o = wpool.tile([128, HID], F32, tag="of")
                        nc.vector.tensor_tensor(o[:], t2[:], bc[:, HID:], OP.subtract)
                        if dbg_apply and l < L - 1:
                            nc.vector.tensor_scalar(o[:], o[:], 0.0, None, OP.max)
                    nc.sync.dma_start(dst[b * 128:(b + 1) * 128, :], o[:])

                if debug == 2 + 2 * l:
                    break
                # ---- allgather between layers
                if l < L - 1:
                    nc.gpsimd.collective_compute(
                        "AllGather", OP.bypass,
                        replica_groups=[list(range(NCORES))],
                        ins=[h_shard[l][:]],
                        outs=[h_full[l][:]],
                    )

    nc.compile()
    return nc


# ----------------------------------------------------------------------------
# public entry point
# ----------------------------------------------------------------------------

def _install_ntff_shim():
    """Optional: register the axon NTFF profiling hook so trace=True works."""
    import types, ctypes, contextlib, os
    if "antenv.axon_hooks" in sys.modules:
        return True
    so_path = "/opt/axon/libaxon_pjrt.so"
    if not os.path.exists(so_path):
        return False
    mod = types.ModuleType("antenv.axon_hooks")
    mod._hook = None
    mod.set_axon_ntff_profile_hook = lambda h: setattr(mod, "_hook", h)
    mod.get_axon_ntff_profile_hook = lambda: mod._hook
    sys.modules["antenv.axon_hooks"] = mod
    try:
        import antenv
        antenv.axon_hooks = mod
    except ImportError:
        pass
    lib = ctypes.CDLL(so_path)
    if not hasattr(lib, "axon_start_nrt_profile"):
        return False
    lib.axon_start_nrt_profile.argtypes = [ctypes.POINTER(ctypes.c_int64), ctypes.c_size_t]
    lib.axon_start_nrt_profile.restype = ctypes.c_int64
    lib.axon_stop_nrt_profile.argtypes = [ctypes.c_char_p]
    lib.axon_stop_nrt_profile.restype = ctypes.c_int64

    @contextlib.contextmanager
    def _hook(output_dir, device_ids):
        import jax
        jax.devices()
        if device_ids:
            ids = (ctypes.c_int64 * len(device_ids))(*device_ids)
            rc = lib.axon_start_nrt_profile(ids, len(device_ids))
        else:
            rc = lib.axon_start_nrt_profile(None, 0)
        if rc != 0:
            raise RuntimeError(f"axon_start_nrt_profile rc={rc}")
        try:
            yield
        finally:
            n = lib.axon_stop_nrt_profile(str(output_dir).encode())
            print(f"ntff profile: {n} file(s) -> {output_dir}", file=sys.stderr)

    mod.set_axon_ntff_profile_hook(_hook)
    return True


def kernel(x, edge_row, edge_col, edge_val, batch,
           W_in, b_in, Ws, bs, alphas, gammas, betas):
    x = np.asarray(x); batch = np.asarray(batch).astype(np.int64)
    edge_row = np.asarray(edge_row).astype(np.int64)
    edge_col = np.asarray(edge_col).astype(np.int64)
    edge_val = np.asarray(edge_val, dtype=np.float32)
    W_in = np.asarray(W_in, np.float32); b_in = np.asarray(b_in, np.float32)
    Ws = np.asarray(Ws, np.float32); bs = np.asarray(bs, np.float32)
    alphas = np.asarray(alphas, np.float32)
    gammas = np.asarray(gammas, np.float32); betas = np.asarray(betas, np.float32)
    G = 256

    in_maps, meta, perm, nodes_of = _host_prep(
        x, edge_row, edge_col, edge_val, batch,
        W_in, b_in, Ws, bs, alphas, gammas, betas, G)

    import os
    debug = int(os.environ.get("GCN_DEBUG", "0"))
    nc = _build_program(meta, debug=debug)
    trace = bool(os.environ.get("GCN_TRACE")) and _install_ntff_shim()
    res = run_bass_kernel_spmd(nc, in_maps, list(range(NCORES)), trace=trace)
    if trace and res.exec_time_ns is not None:
        print(f"HW exec time: {res.exec_time_ns} ns")

    N = x.shape[0]
    out = np.empty((N, HID), np.float32)
    for c in range(NCORES):
        hc = res.results[c]["out_h"]
        out[nodes_of[c]] = hc[perm[c]]
    return out
